# revision 3
# baseline (speedup 1.0000x reference)
"""Multi-head self-attention (B=4, S=2048, D=512, H=8) on 8 trn2 NeuronCores.

Sharding: core c -> (batch c//2, heads 4*(c%2) .. 4*(c%2)+3)  [batch x head-half].
Each core computes a partial transposed output finalT_c [D, S] =
Wo[:, head_slice] @ ctx_heads.T ; host sums the two partials per batch,
transposes back and adds bo.
"""

import sys

sys.path.insert(0, "/opt/trn_rl_repo")

import functools
from contextlib import ExitStack

import numpy as np

B, S, D, H = 4, 2048, 512, 8
DK = D // H           # 64
HLOC = H // 2         # 4 heads per core
DH = HLOC * DK        # 256 local head dims
ST = S // 128         # 16 s(k) tiles
DT = D // 128         # 4 din tiles
NCH = S // 512        # 4 free-dim chunks of 512
QH = 2                # q halves of 1024
SCALE = 1.0 / float(np.sqrt(DK))


def _build():
    import concourse.tile as tile
    from concourse import bacc, mybir

    f32 = mybir.dt.float32
    nc = bacc.Bacc("TRN2", target_bir_lowering=False, debug=False, num_devices=8)

    xT = nc.dram_tensor("xT", [D, S], f32, kind="ExternalInput").ap()
    wqT = nc.dram_tensor("wqT", [D, DH], f32, kind="ExternalInput").ap()
    wkT = nc.dram_tensor("wkT", [D, DH], f32, kind="ExternalInput").ap()
    wvT = nc.dram_tensor("wvT", [D, DH], f32, kind="ExternalInput").ap()
    woT = nc.dram_tensor("woT", [DH, D], f32, kind="ExternalInput").ap()
    bqv = nc.dram_tensor("bq", [DH], f32, kind="ExternalInput").ap()
    bkv = nc.dram_tensor("bk", [DH], f32, kind="ExternalInput").ap()
    bvv = nc.dram_tensor("bv", [DH], f32, kind="ExternalInput").ap()
    maskb = nc.dram_tensor("maskb", [S], f32, kind="ExternalInput").ap()
    outT = nc.dram_tensor("outT", [D, S], f32, kind="ExternalOutput").ap()

    with tile.TileContext(nc) as tc, ExitStack() as ctx:
        Exp = mybir.ActivationFunctionType.Exp

        consts = ctx.enter_context(tc.tile_pool(name="consts", bufs=1))
        xpool = ctx.enter_context(tc.tile_pool(name="xpool", bufs=1))
        qkpool = ctx.enter_context(tc.tile_pool(name="qkpool", bufs=1))
        vpool = ctx.enter_context(tc.tile_pool(name="vpool", bufs=1))
        cpool = ctx.enter_context(tc.tile_pool(name="cpool", bufs=1))
        eppool = ctx.enter_context(tc.tile_pool(name="eppool", bufs=4))
        nrmpool = ctx.enter_context(tc.tile_pool(name="nrmpool", bufs=2))
        outpool = ctx.enter_context(tc.tile_pool(name="outpool", bufs=2))

        # ---- loads -------------------------------------------------------
        def load(pool, dram_ap, shape, tag):
            t = pool.tile(shape, f32, tag=tag, name=tag)
            nc.sync.dma_start(out=t[:], in_=dram_ap)
            return t

        xt = [load(xpool, xT[128 * t : 128 * (t + 1), :], [128, S], f"xt{t}")
              for t in range(DT)]
        wq = [load(consts, wqT[128 * t : 128 * (t + 1), :], [128, DH], f"wq{t}")
              for t in range(DT)]
        wk = [load(consts, wkT[128 * t : 128 * (t + 1), :], [128, DH], f"wk{t}")
              for t in range(DT)]
        wv = [load(consts, wvT[128 * t : 128 * (t + 1), :], [128, DH], f"wv{t}")
              for t in range(DT)]
        wo = [load(consts, woT[128 * t : 128 * (t + 1), :], [128, D], f"wo{t}")
              for t in range(2)]
        bq_sb = load(consts, bqv.rearrange("(m p) -> p m", p=128), [128, 2], "bq")
        bk_sb = load(consts, bkv.rearrange("(m p) -> p m", p=128), [128, 2], "bk")
        bv_sb = load(consts, bvv.rearrange("(o d) -> o d", o=1), [1, DH], "bv")
        mk_sb = load(consts, maskb.rearrange("(k p) -> p k", p=128), [128, ST], "mk")

        ones_row = consts.tile([1, 128], f32, tag="ones", name="ones")
        nc.vector.memset(ones_row[:], 1.0)

        # ---- projections -------------------------------------------------
        with tc.tile_pool(name="pproj", bufs=2, space="PSUM") as ppool:
            qt, kt = [], []
            for w_tiles, bias, dst in ((wq, bq_sb, qt), (wk, bk_sb, kt)):
                for m in range(2):
                    ps = ppool.tile([128, S], f32, tag="ps", name="ps")
                    for t in range(DT):
                        for c in range(NCH):
                            nc.tensor.matmul(
                                ps[:, 512 * c : 512 * (c + 1)],
                                lhsT=w_tiles[t][:, 128 * m : 128 * (m + 1)],
                                rhs=xt[t][:, 512 * c : 512 * (c + 1)],
                                start=(t == 0), stop=(t == DT - 1),
                            )
                    sb = qkpool.tile([128, S], f32, tag=f"qk{len(qt) + len(kt)}_{m}", name=f"qk{len(qt) + len(kt)}_{m}")
                    nc.vector.tensor_scalar_add(sb[:], ps[:], bias[:, m : m + 1])
                    dst.append(sb)

            # V in natural layout [s, dv] with an appended ones column per head:
            # vp[si] is [128, HLOC*65]; head h occupies cols 65h..65h+63, col
            # 65h+64 is all-ones (yields softmax denominators in PV row 64).
            vp = []
            for si in range(ST):
                psv = ppool.tile([128, DH], f32, tag="ps", name="psv")
                for t in range(DT):
                    nc.tensor.matmul(
                        psv[:],
                        lhsT=xt[t][:, 128 * si : 128 * (si + 1)],
                        rhs=wv[t][:, :],
                        start=(t == 0), stop=False,
                    )
                nc.tensor.matmul(  # + bv broadcast over rows
                    psv[:], lhsT=ones_row[:], rhs=bv_sb[:], start=False, stop=True,
                )
                v = vpool.tile([128, HLOC * 65], f32, tag=f"vp{si}", name=f"vp{si}")
                v3 = v[:].rearrange("p (h e) -> p h e", e=65)
                nc.vector.tensor_copy(
                    v3[:, :, 0:64], psv[:].rearrange("p (h d) -> p h d", d=64)
                )
                nc.vector.memset(v3[:, :, 64:65], 1.0)
                vp.append(v)

        ctxn = [cpool.tile([128, S], f32, tag=f"ctxn{m}", name=f"ctxn{m}") for m in range(2)]

        # ---- attention ---------------------------------------------------
        with tc.tile_pool(name="pscore", bufs=2, space="PSUM") as spool, \
             tc.tile_pool(name="pctx", bufs=1, space="PSUM") as ctxpool:
            for h in range(HLOC):
                m, qb = h // 2, (h % 2) * 64
                ctx_ps = ctxpool.tile([128, S], f32, tag="ctx", name="ctx")
                for k in range(ST):
                    for qh in range(QH):
                        ss = spool.tile([128, 1024], f32, tag="ss", name="ss")
                        for c2 in range(2):
                            nc.tensor.matmul(
                                ss[:, 512 * c2 : 512 * (c2 + 1)],
                                lhsT=kt[m][qb : qb + 64, 128 * k : 128 * (k + 1)],
                                rhs=qt[m][qb : qb + 64,
                                          1024 * qh + 512 * c2 : 1024 * qh + 512 * (c2 + 1)],
                                start=True, stop=True,
                            )
                        ep = eppool.tile([128, 1024], f32, tag="ep", name="ep")
                        nc.scalar.activation(
                            ep[:], ss[:], Exp, bias=mk_sb[:, k : k + 1], scale=SCALE,
                        )
                        for c2 in range(2):
                            nc.tensor.matmul(
                                ctx_ps[0:65,
                                       1024 * qh + 512 * c2 : 1024 * qh + 512 * (c2 + 1)],
                                lhsT=vp[k][:, 65 * h : 65 * h + 65],
                                rhs=ep[:, 512 * c2 : 512 * (c2 + 1)],
                                start=(k == 0), stop=(k == ST - 1),
                            )
                inv = nrmpool.tile([1, S], f32, tag="inv", name="inv")
                nc.vector.reciprocal(inv[:], ctx_ps[64:65, :])
                invb = nrmpool.tile([64, S], f32, tag="invb", name="invb")
                nc.gpsimd.partition_broadcast(invb[:], inv[:], channels=64)
                nc.vector.tensor_mul(
                    ctxn[m][qb : qb + 64, :], ctx_ps[0:64, :], invb[:]
                )

        # ---- output projection ------------------------------------------
        with tc.tile_pool(name="pout", bufs=2, space="PSUM") as opool:
            for m in range(DT):
                po = opool.tile([128, S], f32, tag="po", name="po")
                for t in range(2):
                    for c in range(NCH):
                        nc.tensor.matmul(
                            po[:, 512 * c : 512 * (c + 1)],
                            lhsT=wo[t][:, 128 * m : 128 * (m + 1)],
                            rhs=ctxn[t][:, 512 * c : 512 * (c + 1)],
                            start=(t == 0), stop=(t == 1),
                        )
                ob = outpool.tile([128, S], f32, tag="ob", name="ob")
                nc.scalar.copy(ob[:], po[:])
                nc.sync.dma_start(
                    out=outT[128 * m : 128 * (m + 1), :], in_=ob[:]
                )

    nc.compile()
    return nc


@functools.lru_cache(maxsize=1)
def _compiled():
    return _build()


def _in_maps(x, mask, Wq, bq, Wk, bk, Wv, bv, Wo, bo):
    maps = []
    for c in range(8):
        b, half = c // 2, c % 2
        hs = slice(DH * half, DH * (half + 1))
        maps.append({
            "xT": np.ascontiguousarray(x[b].T),
            "wqT": np.ascontiguousarray(Wq[hs].T),
            "wkT": np.ascontiguousarray(Wk[hs].T),
            "wvT": np.ascontiguousarray(Wv[hs].T),
            "woT": np.ascontiguousarray(Wo[:, hs].T),
            "bq": np.ascontiguousarray(bq[hs]),
            "bk": np.ascontiguousarray(bk[hs]),
            "bv": np.ascontiguousarray(bv[hs]),
            "maskb": np.where(mask[b], 0.0, -1e30).astype(np.float32),
        })
    return maps


def _run(in_maps, trace=False):
    from concourse.bass_utils import run_bass_kernel_spmd

    nc = _compiled()
    return run_bass_kernel_spmd(nc, in_maps, list(range(8)), trace=trace)


def kernel(x, mask, Wq, bq, Wk, bk, Wv, bv, Wo, bo, _trace=False, _res_out=None):
    x = np.asarray(x, dtype=np.float32)
    res = _run(_in_maps(np.asarray(x), np.asarray(mask), np.asarray(Wq),
                        np.asarray(bq), np.asarray(Wk), np.asarray(bk),
                        np.asarray(Wv), np.asarray(bv), np.asarray(Wo),
                        np.asarray(bo)), trace=_trace)
    if _res_out is not None:
        _res_out.append(res)
    out = np.empty((B, S, D), dtype=np.float32)
    for b in range(B):
        pT = res.results[2 * b]["outT"] + res.results[2 * b + 1]["outT"]
        out[b] = pT.T + np.asarray(bo, dtype=np.float32)[None, :]
    return out
